# revision 33
# baseline (speedup 1.0000x reference)
"""Bezier-to-image Gaussian splat kernel for Trainium2 (8 NeuronCores).

Reference computation (per sample b of 256):
    T = warped cubic Bernstein basis (30, 4)
    points = einsum('nk,blkc->blnc', T, x.reshape(B,160,4,2))   # (B,160,30,2)
    gx[b,l,i,n] = exp(-(i/60 - X[b,l,n])^2 / 2e-4)
    out[b,i,j]  = min(sum_{l,n} gx[b,l,i,n]*gy[b,l,j,n], 1)     # (B,60,60)

Strategy: pure data parallel, 32 samples per core.  Per sample the 4800
bezier points are processed in 40 chunks of 128 points (4 curves x 32-row
strips; rows 30/31 of each strip are dead and killed via the iota constant);
d[p,i] = i - 60*X_p is built by one broadcast DVE tensor_tensor reading r
straight from PSUM, the Gaussian is evaluated on ScalarE (Derivative_Erf
LUT = 2/sqrt(pi)*exp(-x^2) in a single batched pass), and the 60x60 image
accumulates on PE as sum_c GxT_c^T @ GyT_c in one PSUM bank.

Pipelining: the r matmuls are emitted two samples ahead so the DVE subtract
never waits on PE, and the (PSUM-dependent) min/scale op for sample b is
emitted one iteration late so its wait on the image matmuls overlaps the
next sample's subtract instead of stalling the in-order DVE queue.
"""

import math

import numpy as np
import orjson

import bass_rust
import concourse.bass as bass
import concourse.mybir as mybir
import concourse.tile as tile
from concourse.bass_utils import run_bass_kernel_spmd

B, L, N, W = 256, 160, 30, 60
NCORES = 8
BC = B // NCORES          # samples per core
ALPHA = 2e-4
KEXP = 1.0 / (W * W * ALPHA)          # exponent scale in cell units: 1/0.72
SDERF = math.sqrt(KEXP)               # Derivative_Erf input scale
DERF_FIX = math.pi / 4.0              # undo (2/sqrt(pi))^2 from Derivative_Erf
CHUNKS = 40                           # 4 curves x 30 samples per chunk
PTS = 128                             # chunk partition dim: p = 32*lg + n
CW = 60                               # width of one chunk's band (= W)

LAST_RESULTS = None  # test harness reads profiling info from here


def _basis_T() -> np.ndarray:
    t = np.arange(N, dtype=np.float32) / np.float32(N)
    t = 2 * t**3 - 3 * t**2 + 2 * t
    t_3_0 = t**3
    t_2_1 = t**2 - t_3_0
    t_1_2 = t_3_0 - 2 * t**2 + t
    t_0_3 = (1 - t) ** 3
    return np.stack([t_3_0, 3 * t_2_1, 3 * t_1_2, t_0_3], axis=1).astype(np.float32)


def _legalize_waits(nc, max_waits: int = 1):
    """Walrus rejects engine instructions carrying more than ~1 sync wait
    ("Too many sync wait commands").  Hoist excess waits onto same-engine
    Drain instructions inserted immediately before the offender."""
    js = orjson.loads(mybir.module_to_json_bytes(nc.m))
    ctr = 0
    for f in js["functions"]:
        for bb in f["blocks"]:
            out = []
            changed = False
            for inst in bb["instructions"]:
                si = inst.get("sync_info")
                waits = si.get("on_wait") if si else None
                if waits and len(waits) > max_waits:
                    keep = waits[:max_waits]
                    for w in waits[max_waits:]:
                        ctr += 1
                        out.append({
                            "debug": inst.get("debug", 0),
                            "engine": inst["engine"],
                            "ins": [], "outs": [],
                            "name": f"waitfix-{ctr}",
                            "opcode": "Drain",
                            "sync_info": {"on_update": [], "on_wait": [w]},
                        })
                    si["on_wait"] = keep
                    changed = True
                out.append(inst)
            if changed:
                bb["instructions"] = out
    if ctr:
        nc.m = bass_rust.module_from_json_bytes(orjson.dumps(js))
    return ctr


def build_program(legalize: bool = True):
    f32 = mybir.dt.float32
    f16 = mybir.dt.float16

    nc = bass.Bass("TRN2", target_bir_lowering=False, debug=False)

    x_t = nc.dram_tensor("x", [BC, L, 8], f32, kind="ExternalInput")
    y_t = nc.dram_tensor("y", [BC, W, W], f32, kind="ExternalOutput")

    # (4, 32) stationary operand: r[m] = sum_k TscT[k,m]*ctrl[k] = 60*X.
    tsc_np = np.zeros((4, 32), dtype=np.float32)
    tsc_np[:, :N] = (W * _basis_T()).T
    tsc_d = nc.inline_tensor(tsc_np, name="tscT")

    # x-side iota (chunk-major band layout): dead rows (n in {30,31} of each
    # 32-strip) get +120 so their distance is >= 60 -> gx = 0, killing the
    # dead rows' contribution to the outer product regardless of the y side.
    iota_np = np.tile(np.arange(CW, dtype=np.float16), (PTS, 1))
    for lg in range(4):
        iota_np[32 * lg + 30 : 32 * lg + 32, :] += np.float16(120.0)
    iota_d = nc.inline_tensor(iota_np, name="iota60")

    # The y-side band is evaluated in 15 slices of 4 cells, laid out
    # [p, (s, u, c)].  For slice s, d = (u-2) - r'' with r'' = r - (4s+2);
    # near cells have |r''| < 6 so fp16 r'' keeps 0.002-cell accuracy, and
    # all operands of the y subtract are innermost-packed fp16 (DVE 2x).
    # The (s, u) column order equals ascending j = 4s+u, so the image
    # matmuls' moving operand just reads strided columns, in order.
    NS, UW = 15, 4
    iota4_np = np.zeros((PTS, UW * CHUNKS), dtype=np.float16)
    for u in range(UW):
        iota4_np[:, u * CHUNKS : (u + 1) * CHUNKS] = np.float16(u - 2)
    iota4_d = nc.inline_tensor(iota4_np, name="iota4")
    # shift constants 4s+2, one per slice, broadcast over c
    shift_np = (4.0 * np.arange(NS, dtype=np.float32) + 2.0)[None, :].repeat(
        PTS, axis=0
    ).astype(np.float32)
    shift_d = nc.inline_tensor(shift_np, name="shifts")

    with tile.TileContext(nc) as tc, tc.tile_pool(name="const", bufs=1) as cpool, \
            tc.tile_pool(name="ctrl", bufs=1) as ctrl_pool, \
            tc.tile_pool(name="outp", bufs=1) as out_pool, \
            tc.tile_pool(name="stage", bufs=1) as stage_pool, \
            tc.tile_pool(name="dwork", bufs=3) as dpool, \
            tc.tile_pool(name="band", bufs=3) as band_pool, \
            tc.tile_pool(name="rpsum", bufs=3, space="PSUM") as rps_pool, \
            tc.tile_pool(name="imgpsum", bufs=2, space="PSUM") as img_pool:

        # Prologue: DMA loads land in staging tiles; DVE copies them into the
        # tiles PE reads (PE LDWEIGHTS tolerates very few sync waits).
        tsc0 = cpool.tile([4, 32], f32, tag="tsc0")
        nc.sync.dma_start(tsc0[:], tsc_d.ap())
        tsc = cpool.tile([4, 32], f32, tag="tsc")
        nc.vector.tensor_copy(tsc[:], tsc0[:])
        iot = cpool.tile([PTS, CW], f16, tag="iota")
        nc.sync.dma_start(iot[:], iota_d.ap())
        iot4 = cpool.tile([PTS, UW * CHUNKS], f16, tag="iota4")
        nc.sync.dma_start(iot4[:], iota4_d.ap())
        shf = cpool.tile([PTS, NS], f32, tag="shifts")
        nc.sync.dma_start(shf[:], shift_d.ap())

        # control points: partition k (4), free = (b, l, coord).  All stage
        # DMAs issue up front (parallel queues); the DVE copy for a stage is
        # emitted lazily before the first sample that needs it.  The first
        # stage covers only 2 samples so the pipeline starts early.
        GRP = 8
        stages = [(0, 1), (1, 2), (2, 4), (4, 8)] + [
            (g, g + GRP) for g in range(GRP, BC, GRP)
        ]
        ct = ctrl_pool.tile([4, BC * 2 * L], f32, tag="ct")
        ct_stage = []
        stage_of = {}
        for si, (b0, b1) in enumerate(stages):
            ct0 = stage_pool.tile([4, (b1 - b0) * 2 * L], f32, tag=f"ct{si}")
            nc.sync.dma_start(
                ct0[:].rearrange("k (b l c) -> k b l c", b=b1 - b0, c=2),
                x_t.ap()[b0:b1].rearrange("b l (k c) -> k b l c", k=4),
            )
            ct_stage.append(ct0)
            for b in range(b0, b1):
                stage_of[b] = si
        ct_v = ct[:].rearrange("k (b c g co) -> k b c g co", b=BC, c=CHUNKS, co=2)

        # all 32 output images live here until the per-group DMAs
        out_all = out_pool.tile([W, BC * W], f32, tag="oall")

        CS_ALL = 2 * CHUNKS
        stages_emitted = [False] * len(stages)
        r_tiles = {}
        img_tiles = {}

        def emit_r(b):
            si = stage_of[b]
            if not stages_emitted[si]:
                stages_emitted[si] = True
                b0 = stages[si][0]
                sz = ct_stage[si].shape[1]
                nc.vector.tensor_copy(
                    ct[:, b0 * 2 * L : b0 * 2 * L + sz], ct_stage[si][:]
                )
            r_ps = rps_pool.tile([PTS, 2 * CHUNKS], f32, tag="rps")
            for lg in range(4):
                nc.tensor.matmul(
                    r_ps[32 * lg : 32 * lg + 32, :],
                    lhsT=tsc[:],
                    rhs=ct_v[:, b : b + 1, :, lg : lg + 1, :],
                    start=True,
                    stop=True,
                    tile_position=(0, 32 * lg),
                )
            r_tiles[b] = r_ps

        def emit_min(b):
            """min(s*img, 1) = 1 - relu(1 - s*img), on ScalarE (which has
            slack; keeping this off the in-order DVE queue avoids stalling
            the next subtract on the image matmuls).  Group DMA when a
            group closes."""
            img = img_tiles.pop(b)
            tmp = dpool.tile([W, W], f32, tag="mintmp")
            nc.scalar.activation(
                tmp[:], img[:],
                mybir.ActivationFunctionType.Relu,
                bias=1.0, scale=-DERF_FIX,
            )
            nc.scalar.activation(
                out_all[:, W * b : W * (b + 1)], tmp[:],
                mybir.ActivationFunctionType.Copy,
                bias=1.0, scale=-1.0,
            )
            if b % GRP == GRP - 1:
                g = b // GRP
                nc.sync.dma_start(
                    y_t.ap()[g * GRP : (g + 1) * GRP].rearrange("b i j -> i b j"),
                    out_all[:, W * GRP * g : W * GRP * (g + 1)]
                    .rearrange("i (b j) -> i b j", b=GRP),
                )

        # software pipeline: r two samples ahead, min one sample behind.
        emit_r(0)
        emit_r(1)

        for b in range(BC):
            if b + 2 < BC:
                emit_r(b + 2)
            r_ps = r_tiles.pop(b)

            # ---- banded distance, fp16; r read straight from PSUM.
            # Free-dim layout (chunk, side, cell): x band of chunk c at cols
            # 120c, y band at 120c+60, matching r's (chunk, coord) interleave.
            dd = band_pool.tile([PTS, 2 * CHUNKS * CW], f16, tag="dd")
            nc.vector.tensor_tensor(
                dd[:].rearrange("p (cs w) -> p cs w", w=CW),
                iot[:].rearrange("p (o w) -> p o w", o=1).broadcast_to(
                    [PTS, CS_ALL, CW]
                ),
                r_ps[:].rearrange("p (cs o) -> p cs o", o=1).broadcast_to(
                    [PTS, CS_ALL, CW]
                ),
                mybir.AluOpType.subtract,
            )

            gg = band_pool.tile([PTS, 2 * CHUNKS * CW], f16, tag="gg")
            nc.scalar.activation(
                gg[:], dd[:],
                mybir.ActivationFunctionType.Derivative_Erf,
                bias=0.0, scale=SDERF,
            )

            # ---- image accumulation: sum_c GxT_c^T @ GyT_c ----
            img = img_pool.tile([W, W], f32, tag="img")
            for c in range(CHUNKS):
                nc.tensor.matmul(
                    img[:],
                    lhsT=gg[:, 2 * CW * c : 2 * CW * c + W],
                    rhs=gg[:, 2 * CW * c + CW : 2 * CW * c + CW + W],
                    start=(c == 0),
                    stop=(c == CHUNKS - 1),
                )
            img_tiles[b] = img

            if b > 0:
                emit_min(b - 1)
        emit_min(BC - 1)

    if legalize:
        _legalize_waits(nc)
    return nc


_PROGRAM = None


def kernel(x: np.ndarray, _trace: bool = False) -> np.ndarray:
    global _PROGRAM, LAST_RESULTS
    assert x.shape == (B, L, 8) and x.dtype == np.float32, (x.shape, x.dtype)
    if _PROGRAM is None:
        _PROGRAM = build_program()
    nc = _PROGRAM
    shards = np.split(np.ascontiguousarray(x), NCORES, axis=0)
    in_maps = [{"x": s} for s in shards]
    res = run_bass_kernel_spmd(nc, in_maps, list(range(NCORES)), trace=_trace)
    LAST_RESULTS = res
    return np.concatenate([res.results[i]["y"] for i in range(NCORES)], axis=0)


# revision 38
# speedup vs baseline: 1.1506x; 1.1506x over previous
"""Bezier-to-image Gaussian splat kernel for Trainium2 (8 NeuronCores).

Reference computation (per sample b of 256):
    T = warped cubic Bernstein basis (30, 4)
    points = einsum('nk,blkc->blnc', T, x.reshape(B,160,4,2))   # (B,160,30,2)
    gx[b,l,i,n] = exp(-(i/60 - X[b,l,n])^2 / 2e-4)
    out[b,i,j]  = min(sum_{l,n} gx[b,l,i,n]*gy[b,l,j,n], 1)     # (B,60,60)

Strategy: pure data parallel, 32 samples per core.  Per sample the 4800
bezier points are processed in 40 chunks of 128 points (4 curves x 32-row
strips; rows 30/31 of each strip are dead and killed via the iota constant);
d[p,i] = i - 60*X_p is built by one broadcast DVE tensor_tensor reading r
straight from PSUM, the Gaussian is evaluated on ScalarE (Derivative_Erf
LUT = 2/sqrt(pi)*exp(-x^2) in a single batched pass), and the 60x60 image
accumulates on PE as sum_c GxT_c^T @ GyT_c in one PSUM bank.

Pipelining: the r matmuls are emitted two samples ahead so the DVE subtract
never waits on PE, and the (PSUM-dependent) min/scale op for sample b is
emitted one iteration late so its wait on the image matmuls overlaps the
next sample's subtract instead of stalling the in-order DVE queue.
"""

import math

import numpy as np
import orjson

import bass_rust
import concourse.bass as bass
import concourse.mybir as mybir
import concourse.tile as tile
from concourse.bass_utils import run_bass_kernel_spmd

B, L, N, W = 256, 160, 30, 60
NCORES = 8
BC = B // NCORES          # samples per core
ALPHA = 2e-4
KEXP = 1.0 / (W * W * ALPHA)          # exponent scale in cell units: 1/0.72
SDERF = math.sqrt(KEXP)               # Derivative_Erf input scale
DERF_FIX = math.pi / 4.0              # undo (2/sqrt(pi))^2 from Derivative_Erf
CHUNKS = 40                           # 4 curves x 30 samples per chunk
PTS = 128                             # chunk partition dim: p = 32*lg + n
CW = 60                               # width of one chunk's band (= W)

LAST_RESULTS = None  # test harness reads profiling info from here


def _basis_T() -> np.ndarray:
    t = np.arange(N, dtype=np.float32) / np.float32(N)
    t = 2 * t**3 - 3 * t**2 + 2 * t
    t_3_0 = t**3
    t_2_1 = t**2 - t_3_0
    t_1_2 = t_3_0 - 2 * t**2 + t
    t_0_3 = (1 - t) ** 3
    return np.stack([t_3_0, 3 * t_2_1, 3 * t_1_2, t_0_3], axis=1).astype(np.float32)


def _legalize_waits(nc, max_waits: int = 1):
    """Walrus rejects engine instructions carrying more than ~1 sync wait
    ("Too many sync wait commands").  Hoist excess waits onto same-engine
    Drain instructions inserted immediately before the offender."""
    js = orjson.loads(mybir.module_to_json_bytes(nc.m))
    ctr = 0
    for f in js["functions"]:
        for bb in f["blocks"]:
            out = []
            changed = False
            for inst in bb["instructions"]:
                si = inst.get("sync_info")
                waits = si.get("on_wait") if si else None
                if waits and len(waits) > max_waits:
                    keep = waits[:max_waits]
                    for w in waits[max_waits:]:
                        ctr += 1
                        out.append({
                            "debug": inst.get("debug", 0),
                            "engine": inst["engine"],
                            "ins": [], "outs": [],
                            "name": f"waitfix-{ctr}",
                            "opcode": "Drain",
                            "sync_info": {"on_update": [], "on_wait": [w]},
                        })
                    si["on_wait"] = keep
                    changed = True
                out.append(inst)
            if changed:
                bb["instructions"] = out
    if ctr:
        nc.m = bass_rust.module_from_json_bytes(orjson.dumps(js))
    return ctr


def build_program(legalize: bool = True):
    f32 = mybir.dt.float32
    f16 = mybir.dt.float16

    nc = bass.Bass("TRN2", target_bir_lowering=False, debug=False)

    x_t = nc.dram_tensor("x", [BC, L, 8], f32, kind="ExternalInput")
    y_t = nc.dram_tensor("y", [BC, W, W], f32, kind="ExternalOutput")

    # (4, 32) stationary operand: r[m] = sum_k TscT[k,m]*ctrl[k] = 60*X.
    tsc_np = np.zeros((4, 32), dtype=np.float32)
    tsc_np[:, :N] = (W * _basis_T()).T
    tsc_d = nc.inline_tensor(tsc_np, name="tscT")

    # x-side iota (chunk-major band layout): dead rows (n in {30,31} of each
    # 32-strip) get +120 so their distance is >= 60 -> gx = 0, killing the
    # dead rows' contribution to the outer product regardless of the y side.
    iota_np = np.tile(np.arange(CW, dtype=np.float16), (PTS, 1))
    for lg in range(4):
        iota_np[32 * lg + 30 : 32 * lg + 32, :] += np.float16(120.0)
    iota_d = nc.inline_tensor(iota_np, name="iota60")

    # The y-side band is evaluated in 15 slices of 4 cells, laid out
    # [p, (s, u, c)].  For slice s, d = (u-2) - r'' with r'' = r - (4s+2);
    # near cells have |r''| < 6 so fp16 r'' keeps 0.002-cell accuracy, and
    # all operands of the y subtract are innermost-packed fp16 (DVE 2x).
    # The (s, u) column order equals ascending j = 4s+u, so the image
    # matmuls' moving operand just reads strided columns, in order.
    NS, UW = 15, 4
    iota4_np = np.zeros((PTS, UW * CHUNKS), dtype=np.float16)
    for u in range(UW):
        iota4_np[:, u * CHUNKS : (u + 1) * CHUNKS] = np.float16(u - 2)
    iota4_d = nc.inline_tensor(iota4_np, name="iota4")
    # shift constants 4s+2, one per slice, broadcast over c
    shift_np = (4.0 * np.arange(NS, dtype=np.float32) + 2.0)[None, :].repeat(
        PTS, axis=0
    ).astype(np.float32)
    shift_d = nc.inline_tensor(shift_np, name="shifts")

    with tile.TileContext(nc) as tc, tc.tile_pool(name="const", bufs=1) as cpool, \
            tc.tile_pool(name="ctrl", bufs=1) as ctrl_pool, \
            tc.tile_pool(name="outp", bufs=1) as out_pool, \
            tc.tile_pool(name="stage", bufs=1) as stage_pool, \
            tc.tile_pool(name="dwork", bufs=3) as dpool, \
            tc.tile_pool(name="bsmall", bufs=1) as bsmall_pool, \
            tc.tile_pool(name="band", bufs=2) as band_pool, \
            tc.tile_pool(name="rpsum", bufs=2, space="PSUM") as rps_pool, \
            tc.tile_pool(name="imgpsum", bufs=3, space="PSUM") as img_pool:

        # Prologue: DMA loads land in staging tiles; DVE copies them into the
        # tiles PE reads (PE LDWEIGHTS tolerates very few sync waits).
        tsc0 = cpool.tile([4, 32], f32, tag="tsc0")
        nc.sync.dma_start(tsc0[:], tsc_d.ap())
        tsc = cpool.tile([4, 32], f32, tag="tsc")
        nc.vector.tensor_copy(tsc[:], tsc0[:])
        iot = cpool.tile([PTS, CW], f16, tag="iota")
        nc.sync.dma_start(iot[:], iota_d.ap())
        iot4 = cpool.tile([PTS, UW * CHUNKS], f16, tag="iota4")
        nc.sync.dma_start(iot4[:], iota4_d.ap())
        shf = cpool.tile([PTS, NS], f32, tag="shifts")
        nc.sync.dma_start(shf[:], shift_d.ap())

        # control points: partition k (4), free = (b, l, coord).  All stage
        # DMAs issue up front (parallel queues); the DVE copy for a stage is
        # emitted lazily before the first sample that needs it.  The first
        # stage covers only 2 samples so the pipeline starts early.
        GRP = 8
        stages = [(0, 1), (1, 8)] + [(g, g + GRP) for g in range(GRP, BC, GRP)]
        ct = ctrl_pool.tile([4, BC * 2 * L], f32, tag="ct")
        ct_stage = []
        stage_of = {}
        for si, (b0, b1) in enumerate(stages):
            ct0 = stage_pool.tile([4, (b1 - b0) * 2 * L], f32, tag=f"ct{si}")
            nc.sync.dma_start(
                ct0[:].rearrange("k (b l c) -> k b l c", b=b1 - b0, c=2),
                x_t.ap()[b0:b1].rearrange("b l (k c) -> k b l c", k=4),
            )
            ct_stage.append(ct0)
            for b in range(b0, b1):
                stage_of[b] = si
        ct_v = ct[:].rearrange("k (b c g co) -> k b c g co", b=BC, c=CHUNKS, co=2)

        # all 32 output images live here until the per-group DMAs
        out_all = out_pool.tile([W, BC * W], f32, tag="oall")

        CS_ALL = 2 * CHUNKS
        stages_emitted = [False] * len(stages)
        r_tiles = {}
        img_tiles = {}

        def ensure_ct(b):
            si = stage_of[b]
            if not stages_emitted[si]:
                stages_emitted[si] = True
                s0 = stages[si][0]
                sz = ct_stage[si].shape[1]
                nc.vector.tensor_copy(
                    ct[:, s0 * 2 * L : s0 * 2 * L + sz], ct_stage[si][:]
                )

        def emit_r(b, nb, tag):
            """r matmuls for samples [b, b+nb) into one PSUM tile."""
            for bb in range(b, b + nb):
                ensure_ct(bb)
            r_ps = rps_pool.tile([PTS, nb * 2 * CHUNKS], f32, tag=tag)
            for lg in range(4):
                nc.tensor.matmul(
                    r_ps[32 * lg : 32 * lg + 32, :],
                    lhsT=tsc[:],
                    rhs=ct_v[:, b : b + nb, :, lg : lg + 1, :],
                    start=True,
                    stop=True,
                    tile_position=(0, 32 * lg),
                )
            r_tiles[b] = r_ps

        def emit_min(b):
            """min(s*img, 1) = 1 - relu(1 - s*img), on ScalarE (which has
            slack; keeping this off the in-order DVE queue avoids stalling
            the next subtract on the image matmuls).  Group DMA when a
            group closes."""
            img = img_tiles.pop(b)
            tmp = dpool.tile([W, W], f32, tag="mintmp")
            nc.scalar.activation(
                tmp[:], img[:],
                mybir.ActivationFunctionType.Relu,
                bias=1.0, scale=-DERF_FIX,
            )
            nc.scalar.activation(
                out_all[:, W * b : W * (b + 1)], tmp[:],
                mybir.ActivationFunctionType.Copy,
                bias=1.0, scale=-1.0,
            )
            if b >= BC - GRP:
                # last group: per-sample DMAs so the final transfer is tiny
                nc.sync.dma_start(
                    y_t.ap()[b : b + 1].rearrange("b i j -> i b j"),
                    out_all[:, W * b : W * (b + 1)]
                    .rearrange("i (b j) -> i b j", b=1),
                )
            elif b % GRP == GRP - 1:
                g = b // GRP
                nc.sync.dma_start(
                    y_t.ap()[g * GRP : (g + 1) * GRP].rearrange("b i j -> i b j"),
                    out_all[:, W * GRP * g : W * GRP * (g + 1)]
                    .rearrange("i (b j) -> i b j", b=GRP),
                )

        SAMP = 2 * CHUNKS * CW          # band elems per sample

        def emit_band(r_ps, nb, dd, gg, off):
            """subtract + Gaussian for nb samples into dd/gg at offset."""
            nc.vector.tensor_tensor(
                dd[:, off : off + nb * SAMP].rearrange(
                    "p (b2 cs w) -> p b2 cs w", b2=nb, w=CW
                ),
                iot[:].rearrange("p (o q w) -> p o q w", o=1, q=1).broadcast_to(
                    [PTS, nb, CS_ALL, CW]
                ),
                r_ps[:].rearrange("p (b2 cs o) -> p b2 cs o", b2=nb, o=1)
                .broadcast_to([PTS, nb, CS_ALL, CW]),
                mybir.AluOpType.subtract,
            )
            nc.scalar.activation(
                gg[:, off : off + nb * SAMP], dd[:, off : off + nb * SAMP],
                mybir.ActivationFunctionType.Derivative_Erf,
                bias=0.0, scale=SDERF,
            )

        def emit_img(gg, off, b):
            img = img_pool.tile([W, W], f32, tag="img")
            for c in range(CHUNKS):
                nc.tensor.matmul(
                    img[:],
                    lhsT=gg[:, off + 2 * CW * c : off + 2 * CW * c + W],
                    rhs=gg[:, off + 2 * CW * c + CW : off + 2 * CW * c + CW + W],
                    start=(c == 0),
                    stop=(c == CHUNKS - 1),
                )
            img_tiles[b] = img

        # software pipeline: r one pair ahead, min two samples behind.
        # Samples 0 and 1 run individually (their ct stages land separately,
        # so sample 0 starts as early as possible); the rest run as pairs
        # sharing one subtract + one activation to amortise fixed costs.
        emit_r(0, 1, "rps")
        emit_r(1, 1, "rps")
        dd0 = bsmall_pool.tile([PTS, 2 * SAMP], f16, tag="dd0")
        gg0 = bsmall_pool.tile([PTS, 2 * SAMP], f16, tag="gg0")
        emit_band(r_tiles.pop(0), 1, dd0, gg0, 0)
        emit_r(2, 2, "rpp")
        emit_img(gg0, 0, 0)
        emit_band(r_tiles.pop(1), 1, dd0, gg0, SAMP)
        emit_img(gg0, SAMP, 1)

        for pb in range(1, BC // 2):
            b0 = 2 * pb
            if b0 + 2 < BC:
                emit_r(b0 + 2, 2, "rpp")
            r_ps = r_tiles.pop(b0)
            dd = band_pool.tile([PTS, 2 * SAMP], f16, tag="ddp")
            gg = band_pool.tile([PTS, 2 * SAMP], f16, tag="ggp")
            emit_band(r_ps, 2, dd, gg, 0)
            emit_img(gg, 0, b0)
            emit_min(b0 - 2)
            emit_img(gg, SAMP, b0 + 1)
            emit_min(b0 - 1)
        emit_min(BC - 2)
        emit_min(BC - 1)

    if legalize:
        _legalize_waits(nc)
    return nc


_PROGRAM = None


def kernel(x: np.ndarray, _trace: bool = False) -> np.ndarray:
    global _PROGRAM, LAST_RESULTS
    assert x.shape == (B, L, 8) and x.dtype == np.float32, (x.shape, x.dtype)
    if _PROGRAM is None:
        _PROGRAM = build_program()
    nc = _PROGRAM
    shards = np.split(np.ascontiguousarray(x), NCORES, axis=0)
    in_maps = [{"x": s} for s in shards]
    res = run_bass_kernel_spmd(nc, in_maps, list(range(NCORES)), trace=_trace)
    LAST_RESULTS = res
    return np.concatenate([res.results[i]["y"] for i in range(NCORES)], axis=0)
